# revision 11
# baseline (speedup 1.0000x reference)
"""Trainium2 Bass kernel for ABMIL-MoE-LoRA linear layer.

Reference computation (B=4, N=2048, D_IN=D_OUT=4096, E=8, R=16, D_ATT=128):
    base = x @ W.T + bias
    v = tanh(x @ V.T); u = sigmoid(x @ U.T)
    rw = sigmoid((v*u) @ router_W.T)                    # [B,N,E]
    lora = x @ A_e  (per expert)                        # [B,N,E,R]
    out = base + sum_e rw[...,e] * (lora_e @ B_e)

Strategy: data-parallel over the B*N = 8192 tokens across 8 NeuronCores
(1024 tokens/core, weights replicated). Matmuls run on the TensorEngine
with fp32 PSUM accumulation. Host-side prep pre-transposes every operand
so the contraction dim lands on SBUF partitions.

Precision split: 26 of the 32 contraction k-tiles of the base matmul run
in bf16; the last 6 run as 3 fp8-e4m3 DoubleRow matmuls (2 k-tiles per
pass, half the PE time). The fp8 operands carry cancelling power-of-2
scales (x/8, W*8) so their partial products accumulate into the SAME
PSUM bank as the bf16 partials with no epilogue fixup. Measured rel err
of the hybrid ~1.7e-2 vs the 2e-2 gate.

Schedule: the router/LoRA-down projections are interleaved into the first
two output-column sweeps (k-tile by k-tile, matching DMA arrival order) so
the TensorEngine never starves while x / weights stream in. Those two
sweeps accumulate base-matmul partials into SBUF (PSUM banks are the
scarce resource); later sweeps use the classic 8-bank PSUM accumulation
with the MoE up-projection matmul fused into the same accumulation group.
A burst of dummy matmuls on a memset tile right after the NEFF preamble
warms the PE HAM clock gate so real matmuls never run at K=4/8.

Self-contained: hardcodes all shapes; only imports installed packages.
"""

import numpy as np
import ml_dtypes

BF16 = ml_dtypes.bfloat16

# Problem shapes (hardcoded per spec)
B, N, D_IN, D_OUT = 4, 2048, 4096, 4096
E, R, D_ATT = 8, 16, 128
TOKENS = B * N            # 8192
N_CORES = 8
T = TOKENS // N_CORES     # 1024 tokens per core
KT = D_IN // 128          # 32 contraction k-tiles
KTB = 24                  # k-tiles 0..23 in bf16
KT8 = KT - KTB            # k-tiles 24..31 in fp8 DoubleRow
KP8 = KT8 // 2            # 4 DoubleRow passes
OC = 512                  # output-column chunk per PSUM bank
NOC = D_OUT // OC         # 8 o-chunks
TT = T // 128             # 8 token tiles per core
KH = 2                    # bf16 weight streamed in 2 k-halves
KHT = KTB // KH           # 12 bf16 k-tiles per half
X8S = 8.0                 # x scaled by 1/X8S, W by X8S for the fp8 split
N_WARMUP = 10             # dummy MMs to warm the PE HAM clock gate

_CACHE = {}


def _get_nc():
    if "nc" in _CACHE:
        return _CACHE["nc"]

    import concourse.tile as tile
    import concourse.mybir as mybir
    from concourse import bacc

    dt = mybir.dt
    AFT = mybir.ActivationFunctionType
    nc = bacc.Bacc("TRN2", target_bir_lowering=False, debug=False)

    xT = nc.declare_dram_parameter("xT", [KTB * 128, T], dt.bfloat16, isOutput=False)
    wT = nc.declare_dram_parameter("wT", [KTB * 128, D_OUT], dt.bfloat16, isOutput=False)
    x8bT = nc.declare_dram_parameter("x8bT", [KT8 * 128, T], dt.float8e4, isOutput=False)
    w8T = nc.declare_dram_parameter("w8T", [KT8 * 128, D_OUT], dt.float8e4, isOutput=False)
    projT = nc.declare_dram_parameter("projT", [D_IN, 384], dt.float8e4, isOutput=False)
    xR8 = nc.declare_dram_parameter("xR8", [D_IN, T], dt.float8e4, isOutput=False)
    rwrep = nc.declare_dram_parameter("rwrep", [128, 128], dt.bfloat16, isOutput=False)
    bcat = nc.declare_dram_parameter("bcat", [E * R, D_OUT], dt.bfloat16, isOutput=False)
    biasr = nc.declare_dram_parameter("biasr", [128, D_OUT], dt.bfloat16, isOutput=False)
    out = nc.declare_dram_parameter("out", [T, D_OUT], dt.float32, isOutput=True)

    xT_ap, wT_ap, projT_ap, xR8_ap = xT.ap(), wT.ap(), projT.ap(), xR8.ap()
    x8bT_ap, w8T_ap = x8bT.ap(), w8T.ap()
    rwrep_ap, bcat_ap, biasr_ap, out_ap = rwrep.ap(), bcat.ap(), biasr.ap(), out.ap()

    with tile.TileContext(nc) as tc:
        with (
            tc.tile_pool(name="xpool", bufs=1) as xpool,
            tc.tile_pool(name="wpool", bufs=3) as wpool,
            tc.tile_pool(name="w8pool", bufs=2) as w8pool,
            tc.tile_pool(name="w0pool", bufs=1) as w0pool,
            tc.tile_pool(name="const", bufs=1) as constp,
            tc.tile_pool(name="inter", bufs=1) as inter,
            tc.tile_pool(name="opool", bufs=3) as opool,
            tc.tile_pool(name="ps", bufs=8, space="PSUM") as psp,
        ):
            xsb = xpool.tile([128, KTB * T], dt.bfloat16, tag="xsb")
            vub = inter.tile([128, T], dt.bfloat16, tag="vub")
            rwb = inter.tile([128, T], dt.bfloat16, tag="rwb")
            wtb = inter.tile([128, T], dt.bfloat16, tag="wtb")

            def ps_tile(name):
                return psp.tile([128, 512], dt.float32, tag="ps", name=name)

            # ---- PE warmup: dummy matmuls on a memset tile so the HAM
            # clock gate reaches K=8/8 before the first data-dependent
            # matmul issues (~12us in, right when the first DMAs land).
            # WAW on the single psum tile keeps them serialized. ----
            wub = constp.tile([128, 512], dt.bfloat16, tag="wub")
            nc.vector.memset(wub[:], 1.0)
            wups = ps_tile("warmup")
            for _ in range(N_WARMUP):
                nc.tensor.matmul(wups[:], wub[:, 0:128], wub[:], start=True, stop=True)

            # ---- sweeps 0 and 1: router half-sweep h fused with the base
            # matmul for o-chunk 0, token-half h. The oc0 weight chunk stays
            # resident across both sweeps; each (t) runs one full PSUM
            # accumulation group (24 bf16 k-tiles + 4 fp8 DoubleRow passes
            # + the fused MoE finish matmul).
            # Pointwise DMA demand stays under the HBM limit so the
            # TensorEngine never starves while x streams in. ----
            w0sb = w0pool.tile([128, KTB * OC], dt.bfloat16, tag="w0sb")
            w8sb0 = w0pool.tile([128, KT8 * OC], dt.float8e4, tag="w8sb0")
            x8b = w0pool.tile([128, KT8 * T], dt.float8e4, tag="x8b")
            projsb = w0pool.tile([128, KT * 384], dt.float8e4, tag="projsb")
            xsb8 = w0pool.tile([128, KT * 512], dt.float8e4, tag="xsb8")
            xT_r = xT_ap.rearrange("(a p) t -> p a t", p=128)
            wT_r = wT_ap.rearrange("(a p) o -> p a o", p=128)
            x8bT_r = x8bT_ap.rearrange("(a p) t -> p a t", p=128)
            w8T_r = w8T_ap.rearrange("(a p) o -> p a o", p=128)
            projT_r = projT_ap.rearrange("(a p) c -> p a c", p=128)
            xsb_r = xsb.rearrange("p (a t) -> p a t", a=KTB)
            w0sb_r = w0sb.rearrange("p (a o) -> p a o", a=KTB)
            w8sb0_r = w8sb0.rearrange("p (a o) -> p a o", a=KT8)
            x8b_r = x8b.rearrange("p (a t) -> p a t", a=KT8)
            projsb_r = projsb.rearrange("p (a c) -> p a c", a=KT)
            xR8_r = xR8_ap.rearrange("(a p) t -> p a t", p=128)
            xsb8_r = xsb8.rearrange("p (a t) -> p a t", a=KT)
            ocs0 = slice(0, OC)

            for h, trange in ((0, range(0, 4)), (1, range(4, 8))):
                # all DMAs for this sweep upfront, in consumption order and
                # batched 4 k-tiles per transfer (~0.6us engine issue cost
                # per DMA trigger caps per-ring bandwidth). h0 splits the
                # load across three rings: sync carries the router-critical
                # xsb8/proj stream + late constants, scalar the proj
                # singles/late batches, gpsimd the base x/w bulk.
                hs = slice(h * 512, (h + 1) * 512)
                if h == 0:
                    nc.sync.dma_start(xsb8_r[:, 0:2, :], xR8_r[:, 0:2, hs])
                    nc.scalar.dma_start(projsb_r[:, 0:2, :], projT_r[:, 0:2, :])
                    nc.gpsimd.dma_start(xsb8_r[:, 2:4, :], xR8_r[:, 2:4, hs])
                    for k in range(4, 8, 2):
                        ka = slice(k, k + 2)
                        nc.sync.dma_start(xsb8_r[:, ka, :], xR8_r[:, ka, hs])
                        nc.sync.dma_start(projsb_r[:, ka, :], projT_r[:, ka, :])
                    nc.scalar.dma_start(projsb_r[:, 2:4, :], projT_r[:, 2:4, :])
                    for k in range(0, 8, 2):
                        ka = slice(k, k + 2)
                        nc.scalar.dma_start(xsb_r[:, ka, hs], xT_r[:, ka, hs])
                        nc.scalar.dma_start(w0sb_r[:, ka, :], wT_r[:, ka, ocs0])
                    for k0 in (8, 12):
                        ka = slice(k0, k0 + 4)
                        nc.sync.dma_start(xsb8_r[:, ka, :], xR8_r[:, ka, hs])
                        nc.sync.dma_start(projsb_r[:, ka, :], projT_r[:, ka, :])
                    for k0 in range(16, KT, 4):
                        ka = slice(k0, k0 + 4)
                        nc.sync.dma_start(xsb8_r[:, ka, :], xR8_r[:, ka, hs])
                        nc.scalar.dma_start(projsb_r[:, ka, :], projT_r[:, ka, :])
                    for k0 in range(8, KTB, 4):
                        ka = slice(k0, k0 + 4)
                        nc.gpsimd.dma_start(xsb_r[:, ka, hs], xT_r[:, ka, hs])
                        nc.gpsimd.dma_start(w0sb_r[:, ka, :], wT_r[:, ka, ocs0])
                    # late, non-critical sync tail: fp8 base operands and
                    # constants consumed only in this sweep's ending phase,
                    # kept out of the bandwidth-saturated early stream.
                    nc.sync.dma_start(x8b_r[:, :, hs], x8bT_r[:, :, hs])
                    nc.sync.dma_start(w8sb0_r[:, :, :], w8T_r[:, :, ocs0])
                    rwrepsb = constp.tile([128, 128], dt.bfloat16, tag="rwrepsb")
                    nc.sync.dma_start(rwrepsb[:], rwrep_ap[:])
                    bcatsb = constp.tile([128, D_OUT], dt.bfloat16, tag="bcatsb")
                    nc.sync.dma_start(bcatsb[:], bcat_ap[:])
                    biassb = constp.tile([128, D_OUT], dt.bfloat16, tag="biassb")
                    nc.sync.dma_start(biassb[:], biasr_ap[:])
                else:
                    for k0 in range(0, KT, 4):
                        ka = slice(k0, k0 + 4)
                        nc.sync.dma_start(xsb8_r[:, ka, :], xR8_r[:, ka, hs])
                        if k0 + 4 <= KTB:
                            nc.sync.dma_start(xsb_r[:, ka, hs], xT_r[:, ka, hs])
                    nc.sync.dma_start(x8b_r[:, :, hs], x8bT_r[:, :, hs])

                vps = ps_tile(f"vps{h}")
                ups = ps_tile(f"ups{h}")
                lps = ps_tile(f"lps{h}")
                pst = {t: ps_tile(f"pst0_{t}") for t in trange}
                DELAY = 6

                def base_mms(k, trange=trange, pst=pst):
                    for t in trange:
                        nc.tensor.matmul(
                            pst[t][:],
                            xsb[:, k * T + t * 128 : k * T + (t + 1) * 128],
                            w0sb[:, k * OC : (k + 1) * OC],
                            start=(k == 0),
                            stop=False,
                        )

                DR = mybir.MatmulPerfMode.DoubleRow
                for k in range(KT):
                    if k % 2 == 0:
                        kp = k // 2
                        st, sp = kp == 0, kp == KT // 2 - 1
                        kpair = slice(k, k + 2)
                        rx8 = xsb8_r[:, kpair, :]
                        pj = projsb_r[:, kpair, :]
                        nc.tensor.matmul(
                            vps[:], pj[:, :, 0:128], rx8,
                            start=st, stop=sp, perf_mode=DR,
                        )
                        nc.tensor.matmul(
                            ups[:], pj[:, :, 128:256], rx8,
                            start=st, stop=sp, perf_mode=DR,
                        )
                        nc.tensor.matmul(
                            lps[:], pj[:, :, 256:384], rx8,
                            start=st, stop=sp, perf_mode=DR,
                        )
                    if k >= DELAY and k - DELAY < KTB:
                        base_mms(k - DELAY)
                for k in range(KT - DELAY, KTB):
                    base_mms(k)

                # router epilogue for half h (the ABMIL gate is per-token
                # separable, so scores/gates/weighted-lora complete within
                # the half). The fp8 DoubleRow tails interleave around the
                # scores matmul to cover the scalar/vector latency chain.
                vtmp = inter.tile([128, 512], dt.float32, tag="vtmp", name=f"vtmp{h}")
                utmp = inter.tile([128, 512], dt.float32, tag="utmp", name=f"utmp{h}")
                nc.scalar.activation(vtmp[:], vps[:], AFT.Tanh, scale=1.0 / 64)
                nc.scalar.activation(utmp[:], ups[:], AFT.Sigmoid, scale=1.0 / 64)
                nc.vector.tensor_mul(vub[:, hs], vtmp[:], utmp[:])

                def dr_tail(t):
                    for p in range(KP8):
                        pr = slice(2 * p, 2 * p + 2)
                        nc.tensor.matmul(
                            pst[t][:],
                            x8b_r[:, pr, t * 128 : (t + 1) * 128],
                            w8sb0_r[:, pr, :],
                            start=False,
                            stop=False,
                            perf_mode=DR,
                        )

                ts_ = list(trange)
                dr_tail(ts_[0])
                dr_tail(ts_[1])
                sps = ps_tile(f"sps{h}")
                nc.tensor.matmul(sps[:], rwrepsb[:], vub[:, hs], start=True, stop=True)
                nc.scalar.activation(rwb[:, hs], sps[:], AFT.Sigmoid)
                nc.vector.tensor_mul(wtb[:, hs], lps[:], rwb[:, hs])
                dr_tail(ts_[2])
                dr_tail(ts_[3])
                for t in trange:
                    nc.tensor.matmul(
                        pst[t][:],
                        wtb[:, t * 128 : (t + 1) * 128],
                        bcatsb[:, ocs0],
                        start=False,
                        stop=True,
                    )
                    osb = opool.tile([128, 512], dt.float32, tag="osb")
                    nc.vector.tensor_add(osb[:], pst[t][:], biassb[:, ocs0])
                    nc.gpsimd.dma_start(
                        out_ap[t * 128 : (t + 1) * 128, ocs0], osb[:]
                    )

            # ---- sweeps 1..7: classic 8-bank PSUM accumulation with the
            # MoE up-projection fused into each group. sync carries only the
            # input weight stream (so prefetch never queues behind output
            # triggers); gpsimd carries the output stream. ----
            DRm = mybir.MatmulPerfMode.DoubleRow

            def classic_sweep(oc):
                ocs = slice(oc * OC, (oc + 1) * OC)
                pst = [None] * TT
                w8sb = w8pool.tile(
                    [128, KT8 * OC], dt.float8e4, tag="w8sb", name=f"w8sb{oc}"
                )
                w8sb_r = w8sb.rearrange("p (a o) -> p a o", a=KT8)
                nc.sync.dma_start(w8sb_r[:, :, :], w8T_r[:, :, ocs])
                for kh in range(KH):
                    wsb = wpool.tile(
                        [128, KHT * OC], dt.bfloat16, tag="wsb", name=f"wsb{oc}_{kh}"
                    )
                    wsb_r = wsb.rearrange("p (a o) -> p a o", a=KHT)
                    for kk0 in range(0, KHT, 4):
                        kk1 = min(kk0 + 4, KHT)
                        nc.sync.dma_start(
                            wsb_r[:, kk0:kk1, :],
                            wT_r[:, kh * KHT + kk0 : kh * KHT + kk1, ocs],
                        )
                    for t in range(TT):
                        if kh == 0:
                            pst[t] = ps_tile(f"pst{oc}_{t}")
                            # fp8 DoubleRow passes open the group (their
                            # small weight DMA lands before the bf16 bulk)
                            for p in range(KP8):
                                pr = slice(2 * p, 2 * p + 2)
                                nc.tensor.matmul(
                                    pst[t][:],
                                    x8b_r[:, pr, t * 128 : (t + 1) * 128],
                                    w8sb_r[:, pr, :],
                                    start=(p == 0),
                                    stop=False,
                                    perf_mode=DRm,
                                )
                        for kk in range(KHT):
                            k = kh * KHT + kk
                            nc.tensor.matmul(
                                pst[t][:],
                                xsb[:, k * T + t * 128 : k * T + (t + 1) * 128],
                                wsb[:, kk * OC : (kk + 1) * OC],
                                start=False,
                                stop=False,
                            )
                        if kh == KH - 1:
                            nc.tensor.matmul(
                                pst[t][:],
                                wtb[:, t * 128 : (t + 1) * 128],
                                bcatsb[:, ocs],
                                start=False,
                                stop=True,
                            )
                            osb = opool.tile([128, 512], dt.float32, tag="osb")
                            if oc == NOC - 1 and t == TT - 1:
                                # final tile: split add+store across two DMA
                                # queues to halve the tail latency
                                o0 = oc * OC
                                nc.vector.tensor_add(
                                    osb[:, 0:256], pst[t][:, 0:256],
                                    biassb[:, o0 : o0 + 256],
                                )
                                nc.sync.dma_start(
                                    out_ap[t * 128 :, o0 : o0 + 256],
                                    osb[:, 0:256],
                                )
                                nc.vector.tensor_add(
                                    osb[:, 256:512], pst[t][:, 256:512],
                                    biassb[:, o0 + 256 : o0 + 512],
                                )
                                nc.scalar.dma_start(
                                    out_ap[t * 128 :, o0 + 256 : o0 + 512],
                                    osb[:, 256:512],
                                )
                            else:
                                nc.vector.tensor_add(osb[:], pst[t][:], biassb[:, ocs])
                                nc.gpsimd.dma_start(
                                    out_ap[t * 128 : (t + 1) * 128, ocs], osb[:]
                                )

            for oc in range(1, NOC):
                classic_sweep(oc)

    nc.compile()
    _CACHE["nc"] = nc
    return nc


def _prep_in_maps(x, weight, bias, router_V, router_U, router_W, experts_A, experts_B):
    FP8 = ml_dtypes.float8_e4m3
    xT_full = np.ascontiguousarray(
        x.reshape(TOKENS, D_IN).T.astype(np.float32)
    )  # [D_IN, TOKENS] fp32
    KB = KTB * 128
    xT_all = np.ascontiguousarray(xT_full[:KB].astype(BF16))        # bf16 part
    x8bT_all = np.ascontiguousarray((xT_full[KB:] / X8S).astype(FP8))  # fp8 part
    wT_full = weight.T.astype(np.float32)  # [D_IN, D_OUT]
    wT = np.ascontiguousarray(wT_full[:KB].astype(BF16))
    w8T = np.ascontiguousarray((wT_full[KB:] * X8S).astype(FP8))
    # projections pre-scaled x64 into fp8's normal range; the x64 is undone
    # by the activation scale (v, u) and by bcat's /64 (lora path)
    projT = np.ascontiguousarray(
        np.concatenate(
            [
                router_V.T,  # [D_IN, 128]
                router_U.T,  # [D_IN, 128]
                experts_A.transpose(1, 0, 2).reshape(D_IN, E * R),  # [D_IN, 128]
            ],
            axis=1,
        )
        * 64.0
    ).astype(FP8)
    xR8_all = xT_full.astype(FP8)
    rwrep = np.ascontiguousarray(np.repeat(router_W, R, axis=0).T.astype(BF16))
    bcat = np.ascontiguousarray((experts_B.reshape(E * R, D_OUT) / 64.0).astype(BF16))
    biasr = np.ascontiguousarray(
        np.broadcast_to(bias.astype(BF16), (128, D_OUT))
    )

    in_maps = []
    for c in range(N_CORES):
        ts = slice(c * T, (c + 1) * T)
        in_maps.append(
            {
                "xT": np.ascontiguousarray(xT_all[:, ts]),
                "x8bT": np.ascontiguousarray(x8bT_all[:, ts]),
                "xR8": np.ascontiguousarray(xR8_all[:, ts]),
                "wT": wT,
                "w8T": w8T,
                "projT": projT,
                "rwrep": rwrep,
                "bcat": bcat,
                "biasr": biasr,
            }
        )
    return in_maps


def _gather(results):
    out = np.concatenate(
        [np.asarray(results[c]["out"], dtype=np.float32) for c in range(N_CORES)],
        axis=0,
    )
    return out.reshape(B, N, D_OUT)


def kernel(x, weight, bias, router_V, router_U, router_W, experts_A, experts_B):
    import time
    from concourse.bass_utils import run_bass_kernel_spmd

    nc = _get_nc()
    in_maps = _prep_in_maps(
        x, weight, bias, router_V, router_U, router_W, experts_A, experts_B
    )
    last_err = None
    for attempt in range(3):
        try:
            res = run_bass_kernel_spmd(nc, in_maps, list(range(N_CORES)))
            return _gather(res.results)
        except Exception as e:  # transient NRT device errors — retry
            last_err = e
            try:  # drop the (possibly wedged) PJRT device context
                import jax

                jax.clear_caches()
                clear = getattr(
                    getattr(getattr(jax, "extend", None), "backend", None),
                    "clear_backends",
                    None,
                ) or getattr(jax, "clear_backends", None)
                if clear is not None:
                    clear()
            except Exception:
                pass
            time.sleep(5 * (attempt + 1))
    raise last_err


def run_traced(x, weight, bias, router_V, router_U, router_W, experts_A, experts_B):
    """Correctness + HW timing run (profiled). Returns (out, exec_time_ns, trace)."""
    import concourse.bass_utils as bass_utils

    bass_utils.upload_artifacts = lambda tmpdir: tmpdir  # no fileshare here
    nc = _get_nc()
    in_maps = _prep_in_maps(
        x, weight, bias, router_V, router_U, router_W, experts_A, experts_B
    )
    res = bass_utils.run_bass_kernel_spmd(
        nc, in_maps, list(range(N_CORES)), trace=True
    )
    trace_path = None
    if res.instructions_and_trace is not None:
        trace_path = res.instructions_and_trace[1]
    return _gather(res.results), res.exec_time_ns, trace_path


# revision 12
# speedup vs baseline: 1.0326x; 1.0326x over previous
"""Trainium2 Bass kernel for ABMIL-MoE-LoRA linear layer.

Reference computation (B=4, N=2048, D_IN=D_OUT=4096, E=8, R=16, D_ATT=128):
    base = x @ W.T + bias
    v = tanh(x @ V.T); u = sigmoid(x @ U.T)
    rw = sigmoid((v*u) @ router_W.T)                    # [B,N,E]
    lora = x @ A_e  (per expert)                        # [B,N,E,R]
    out = base + sum_e rw[...,e] * (lora_e @ B_e)

Strategy: data-parallel over the B*N = 8192 tokens across 8 NeuronCores
(1024 tokens/core, weights replicated). Matmuls run on the TensorEngine
with fp32 PSUM accumulation. Host-side prep pre-transposes every operand
so the contraction dim lands on SBUF partitions.

Precision split: 26 of the 32 contraction k-tiles of the base matmul run
in bf16; the last 6 run as 3 fp8-e4m3 DoubleRow matmuls (2 k-tiles per
pass, half the PE time). The fp8 operands carry cancelling power-of-2
scales (x/8, W*8) so their partial products accumulate into the SAME
PSUM bank as the bf16 partials with no epilogue fixup. Measured rel err
of the hybrid ~1.7e-2 vs the 2e-2 gate.

Schedule: the router/LoRA-down projections are interleaved into the first
two output-column sweeps (k-tile by k-tile, matching DMA arrival order) so
the TensorEngine never starves while x / weights stream in. Those two
sweeps accumulate base-matmul partials into SBUF (PSUM banks are the
scarce resource); later sweeps use the classic 8-bank PSUM accumulation
with the MoE up-projection matmul fused into the same accumulation group.
A burst of dummy matmuls on a memset tile right after the NEFF preamble
warms the PE HAM clock gate so real matmuls never run at K=4/8.

Self-contained: hardcodes all shapes; only imports installed packages.
"""

import numpy as np
import ml_dtypes

BF16 = ml_dtypes.bfloat16

# Problem shapes (hardcoded per spec)
B, N, D_IN, D_OUT = 4, 2048, 4096, 4096
E, R, D_ATT = 8, 16, 128
TOKENS = B * N            # 8192
N_CORES = 8
T = TOKENS // N_CORES     # 1024 tokens per core
KT = D_IN // 128          # 32 contraction k-tiles
KTB = 24                  # k-tiles 0..23 in bf16
KT8 = KT - KTB            # k-tiles 24..31 in fp8 DoubleRow
KP8 = KT8 // 2            # 4 DoubleRow passes
OC = 512                  # output-column chunk per PSUM bank
NOC = D_OUT // OC         # 8 o-chunks
TT = T // 128             # 8 token tiles per core
KH = 2                    # bf16 weight streamed in 2 k-halves
KHT = KTB // KH           # 12 bf16 k-tiles per half
X8S = 8.0                 # x scaled by 1/X8S, W by X8S for the fp8 split
N_WARMUP = 10             # dummy MMs to warm the PE HAM clock gate

_CACHE = {}


def _get_nc():
    if "nc" in _CACHE:
        return _CACHE["nc"]

    import concourse.tile as tile
    import concourse.mybir as mybir
    from concourse import bacc

    dt = mybir.dt
    AFT = mybir.ActivationFunctionType
    nc = bacc.Bacc("TRN2", target_bir_lowering=False, debug=False)

    xT = nc.declare_dram_parameter("xT", [KTB * 128, T], dt.bfloat16, isOutput=False)
    wT = nc.declare_dram_parameter("wT", [KTB * 128, D_OUT], dt.bfloat16, isOutput=False)
    x8bT = nc.declare_dram_parameter("x8bT", [KT8 * 128, T], dt.float8e4, isOutput=False)
    w8T = nc.declare_dram_parameter("w8T", [KT8 * 128, D_OUT], dt.float8e4, isOutput=False)
    projT = nc.declare_dram_parameter("projT", [D_IN, 384], dt.float8e4, isOutput=False)
    xR8 = nc.declare_dram_parameter("xR8", [D_IN, T], dt.float8e4, isOutput=False)
    rwrep = nc.declare_dram_parameter("rwrep", [128, 128], dt.bfloat16, isOutput=False)
    bcat = nc.declare_dram_parameter("bcat", [E * R, D_OUT], dt.bfloat16, isOutput=False)
    biasr = nc.declare_dram_parameter("biasr", [128, D_OUT], dt.bfloat16, isOutput=False)
    out = nc.declare_dram_parameter("out", [T, D_OUT], dt.float32, isOutput=True)

    xT_ap, wT_ap, projT_ap, xR8_ap = xT.ap(), wT.ap(), projT.ap(), xR8.ap()
    x8bT_ap, w8T_ap = x8bT.ap(), w8T.ap()
    rwrep_ap, bcat_ap, biasr_ap, out_ap = rwrep.ap(), bcat.ap(), biasr.ap(), out.ap()

    with tile.TileContext(nc) as tc:
        with (
            tc.tile_pool(name="xpool", bufs=1) as xpool,
            tc.tile_pool(name="wpool", bufs=3) as wpool,
            tc.tile_pool(name="w8pool", bufs=2) as w8pool,
            tc.tile_pool(name="w0pool", bufs=1) as w0pool,
            tc.tile_pool(name="const", bufs=1) as constp,
            tc.tile_pool(name="inter", bufs=1) as inter,
            tc.tile_pool(name="opool", bufs=3) as opool,
            tc.tile_pool(name="ps", bufs=8, space="PSUM") as psp,
        ):
            xsb = xpool.tile([128, KTB * T], dt.bfloat16, tag="xsb")
            vub = inter.tile([128, T], dt.bfloat16, tag="vub")
            rwb = inter.tile([128, T], dt.bfloat16, tag="rwb")
            wtb = inter.tile([128, T], dt.bfloat16, tag="wtb")

            def ps_tile(name):
                return psp.tile([128, 512], dt.float32, tag="ps", name=name)

            # ---- PE warmup: dummy matmuls on a memset tile so the HAM
            # clock gate reaches K=8/8 before the first data-dependent
            # matmul issues (~12us in, right when the first DMAs land).
            # WAW on the single psum tile keeps them serialized. ----
            wub = constp.tile([128, 512], dt.bfloat16, tag="wub")
            nc.vector.memset(wub[:], 1.0)
            wups = ps_tile("warmup")
            for _ in range(N_WARMUP):
                nc.tensor.matmul(wups[:], wub[:, 0:128], wub[:], start=True, stop=True)

            # ---- sweeps 0 and 1: router half-sweep h fused with the base
            # matmul for o-chunk 0, token-half h. The oc0 weight chunk stays
            # resident across both sweeps; each (t) runs one full PSUM
            # accumulation group (24 bf16 k-tiles + 4 fp8 DoubleRow passes
            # + the fused MoE finish matmul).
            # Pointwise DMA demand stays under the HBM limit so the
            # TensorEngine never starves while x streams in. ----
            w0sb = w0pool.tile([128, KTB * OC], dt.bfloat16, tag="w0sb")
            w8sb0 = w0pool.tile([128, KT8 * OC], dt.float8e4, tag="w8sb0")
            x8b = w0pool.tile([128, KT8 * T], dt.float8e4, tag="x8b")
            projsb = w0pool.tile([128, KT * 384], dt.float8e4, tag="projsb")
            xsb8 = w0pool.tile([128, KT * 512], dt.float8e4, tag="xsb8")
            xT_r = xT_ap.rearrange("(a p) t -> p a t", p=128)
            wT_r = wT_ap.rearrange("(a p) o -> p a o", p=128)
            x8bT_r = x8bT_ap.rearrange("(a p) t -> p a t", p=128)
            w8T_r = w8T_ap.rearrange("(a p) o -> p a o", p=128)
            projT_r = projT_ap.rearrange("(a p) c -> p a c", p=128)
            xsb_r = xsb.rearrange("p (a t) -> p a t", a=KTB)
            w0sb_r = w0sb.rearrange("p (a o) -> p a o", a=KTB)
            w8sb0_r = w8sb0.rearrange("p (a o) -> p a o", a=KT8)
            x8b_r = x8b.rearrange("p (a t) -> p a t", a=KT8)
            projsb_r = projsb.rearrange("p (a c) -> p a c", a=KT)
            xR8_r = xR8_ap.rearrange("(a p) t -> p a t", p=128)
            xsb8_r = xsb8.rearrange("p (a t) -> p a t", a=KT)
            ocs0 = slice(0, OC)

            for h, trange in ((0, range(0, 4)), (1, range(4, 8))):
                # all DMAs for this sweep upfront, in consumption order and
                # batched 4 k-tiles per transfer (~0.6us engine issue cost
                # per DMA trigger). Aggregate HBM bandwidth is the binding
                # constraint early on, and each ring is FIFO — so keep ONE
                # serialized consumption-ordered stream on sync (scalar only
                # covers the first singles so sync's trigger-issue cost is
                # hidden); parallel bulk rings just steal bandwidth from the
                # urgent transfers.
                hs = slice(h * 512, (h + 1) * 512)
                if h == 0:
                    nc.sync.dma_start(xsb8_r[:, 0:2, :], xR8_r[:, 0:2, hs])
                    nc.scalar.dma_start(projsb_r[:, 0:2, :], projT_r[:, 0:2, :])
                    nc.gpsimd.dma_start(xsb8_r[:, 2:4, :], xR8_r[:, 2:4, hs])
                    rwrepsb = constp.tile([128, 128], dt.bfloat16, tag="rwrepsb")
                    nc.gpsimd.dma_start(rwrepsb[:], rwrep_ap[:])
                    for k in range(4, 8, 2):
                        ka = slice(k, k + 2)
                        nc.sync.dma_start(xsb8_r[:, ka, :], xR8_r[:, ka, hs])
                        nc.sync.dma_start(projsb_r[:, ka, :], projT_r[:, ka, :])
                    nc.scalar.dma_start(projsb_r[:, 2:4, :], projT_r[:, 2:4, :])
                    for k in range(0, 8, 2):
                        ka = slice(k, k + 2)
                        nc.scalar.dma_start(xsb_r[:, ka, hs], xT_r[:, ka, hs])
                        nc.scalar.dma_start(w0sb_r[:, ka, :], wT_r[:, ka, ocs0])
                    for k0 in range(8, KT, 4):
                        ka = slice(k0, k0 + 4)
                        nc.sync.dma_start(xsb8_r[:, ka, :], xR8_r[:, ka, hs])
                        nc.sync.dma_start(projsb_r[:, ka, :], projT_r[:, ka, :])
                        if k0 + 4 <= KTB:
                            nc.sync.dma_start(xsb_r[:, ka, hs], xT_r[:, ka, hs])
                            nc.sync.dma_start(w0sb_r[:, ka, :], wT_r[:, ka, ocs0])
                    # late, non-critical sync tail: fp8 base operands and
                    # constants consumed only in this sweep's ending phase,
                    # kept out of the bandwidth-saturated early stream.
                    nc.sync.dma_start(x8b_r[:, :, hs], x8bT_r[:, :, hs])
                    nc.sync.dma_start(w8sb0_r[:, :, :], w8T_r[:, :, ocs0])
                    bcatsb = constp.tile([128, D_OUT], dt.bfloat16, tag="bcatsb")
                    nc.sync.dma_start(bcatsb[:], bcat_ap[:])
                    biassb = constp.tile([128, D_OUT], dt.bfloat16, tag="biassb")
                    nc.sync.dma_start(biassb[:], biasr_ap[:])
                else:
                    for k0 in range(0, KT, 4):
                        ka = slice(k0, k0 + 4)
                        nc.sync.dma_start(xsb8_r[:, ka, :], xR8_r[:, ka, hs])
                        if k0 + 4 <= KTB:
                            nc.sync.dma_start(xsb_r[:, ka, hs], xT_r[:, ka, hs])
                    nc.sync.dma_start(x8b_r[:, :, hs], x8bT_r[:, :, hs])

                vps = ps_tile(f"vps{h}")
                ups = ps_tile(f"ups{h}")
                lps = ps_tile(f"lps{h}")
                pst = {t: ps_tile(f"pst0_{t}") for t in trange}
                DELAY = 6

                def base_mms(k, trange=trange, pst=pst):
                    for t in trange:
                        nc.tensor.matmul(
                            pst[t][:],
                            xsb[:, k * T + t * 128 : k * T + (t + 1) * 128],
                            w0sb[:, k * OC : (k + 1) * OC],
                            start=(k == 0),
                            stop=False,
                        )

                DR = mybir.MatmulPerfMode.DoubleRow
                for k in range(KT):
                    if k % 2 == 0:
                        kp = k // 2
                        st, sp = kp == 0, kp == KT // 2 - 1
                        kpair = slice(k, k + 2)
                        rx8 = xsb8_r[:, kpair, :]
                        pj = projsb_r[:, kpair, :]
                        nc.tensor.matmul(
                            vps[:], pj[:, :, 0:128], rx8,
                            start=st, stop=sp, perf_mode=DR,
                        )
                        nc.tensor.matmul(
                            ups[:], pj[:, :, 128:256], rx8,
                            start=st, stop=sp, perf_mode=DR,
                        )
                        nc.tensor.matmul(
                            lps[:], pj[:, :, 256:384], rx8,
                            start=st, stop=sp, perf_mode=DR,
                        )
                    if k >= DELAY and k - DELAY < KTB:
                        base_mms(k - DELAY)
                for k in range(KT - DELAY, KTB):
                    base_mms(k)

                # router epilogue for half h (the ABMIL gate is per-token
                # separable, so scores/gates/weighted-lora complete within
                # the half). The fp8 DoubleRow tails interleave around the
                # scores matmul to cover the scalar/vector latency chain.
                vtmp = inter.tile([128, 512], dt.float32, tag="vtmp", name=f"vtmp{h}")
                utmp = inter.tile([128, 512], dt.float32, tag="utmp", name=f"utmp{h}")
                nc.scalar.activation(vtmp[:], vps[:], AFT.Tanh, scale=1.0 / 64)
                nc.scalar.activation(utmp[:], ups[:], AFT.Sigmoid, scale=1.0 / 64)
                nc.vector.tensor_mul(vub[:, hs], vtmp[:], utmp[:])

                def dr_tail(t):
                    for p in range(KP8):
                        pr = slice(2 * p, 2 * p + 2)
                        nc.tensor.matmul(
                            pst[t][:],
                            x8b_r[:, pr, t * 128 : (t + 1) * 128],
                            w8sb0_r[:, pr, :],
                            start=False,
                            stop=False,
                            perf_mode=DR,
                        )

                ts_ = list(trange)
                dr_tail(ts_[0])
                dr_tail(ts_[1])
                sps = ps_tile(f"sps{h}")
                nc.tensor.matmul(sps[:], rwrepsb[:], vub[:, hs], start=True, stop=True)
                nc.scalar.activation(rwb[:, hs], sps[:], AFT.Sigmoid)
                nc.vector.tensor_mul(wtb[:, hs], lps[:], rwb[:, hs])
                dr_tail(ts_[2])
                dr_tail(ts_[3])
                for t in trange:
                    nc.tensor.matmul(
                        pst[t][:],
                        wtb[:, t * 128 : (t + 1) * 128],
                        bcatsb[:, ocs0],
                        start=False,
                        stop=True,
                    )
                    osb = opool.tile([128, 512], dt.float32, tag="osb")
                    nc.vector.tensor_add(osb[:], pst[t][:], biassb[:, ocs0])
                    nc.gpsimd.dma_start(
                        out_ap[t * 128 : (t + 1) * 128, ocs0], osb[:]
                    )

            # ---- sweeps 1..7: classic 8-bank PSUM accumulation with the
            # MoE up-projection fused into each group. sync carries only the
            # input weight stream (so prefetch never queues behind output
            # triggers); gpsimd carries the output stream. ----
            DRm = mybir.MatmulPerfMode.DoubleRow

            def classic_sweep(oc):
                ocs = slice(oc * OC, (oc + 1) * OC)
                pst = [None] * TT
                w8sb = w8pool.tile(
                    [128, KT8 * OC], dt.float8e4, tag="w8sb", name=f"w8sb{oc}"
                )
                w8sb_r = w8sb.rearrange("p (a o) -> p a o", a=KT8)
                nc.sync.dma_start(w8sb_r[:, :, :], w8T_r[:, :, ocs])
                for kh in range(KH):
                    wsb = wpool.tile(
                        [128, KHT * OC], dt.bfloat16, tag="wsb", name=f"wsb{oc}_{kh}"
                    )
                    wsb_r = wsb.rearrange("p (a o) -> p a o", a=KHT)
                    for kk0 in range(0, KHT, 4):
                        kk1 = min(kk0 + 4, KHT)
                        nc.sync.dma_start(
                            wsb_r[:, kk0:kk1, :],
                            wT_r[:, kh * KHT + kk0 : kh * KHT + kk1, ocs],
                        )
                    for t in range(TT):
                        if kh == 0:
                            pst[t] = ps_tile(f"pst{oc}_{t}")
                            # fp8 DoubleRow passes open the group (their
                            # small weight DMA lands before the bf16 bulk)
                            for p in range(KP8):
                                pr = slice(2 * p, 2 * p + 2)
                                nc.tensor.matmul(
                                    pst[t][:],
                                    x8b_r[:, pr, t * 128 : (t + 1) * 128],
                                    w8sb_r[:, pr, :],
                                    start=(p == 0),
                                    stop=False,
                                    perf_mode=DRm,
                                )
                        for kk in range(KHT):
                            k = kh * KHT + kk
                            nc.tensor.matmul(
                                pst[t][:],
                                xsb[:, k * T + t * 128 : k * T + (t + 1) * 128],
                                wsb[:, kk * OC : (kk + 1) * OC],
                                start=False,
                                stop=False,
                            )
                        if kh == KH - 1:
                            nc.tensor.matmul(
                                pst[t][:],
                                wtb[:, t * 128 : (t + 1) * 128],
                                bcatsb[:, ocs],
                                start=False,
                                stop=True,
                            )
                            osb = opool.tile([128, 512], dt.float32, tag="osb")
                            if oc == NOC - 1 and t == TT - 1:
                                # final tile: split add+store across two DMA
                                # queues to halve the tail latency
                                o0 = oc * OC
                                nc.vector.tensor_add(
                                    osb[:, 0:256], pst[t][:, 0:256],
                                    biassb[:, o0 : o0 + 256],
                                )
                                nc.sync.dma_start(
                                    out_ap[t * 128 :, o0 : o0 + 256],
                                    osb[:, 0:256],
                                )
                                nc.vector.tensor_add(
                                    osb[:, 256:512], pst[t][:, 256:512],
                                    biassb[:, o0 + 256 : o0 + 512],
                                )
                                nc.scalar.dma_start(
                                    out_ap[t * 128 :, o0 + 256 : o0 + 512],
                                    osb[:, 256:512],
                                )
                            else:
                                nc.vector.tensor_add(osb[:], pst[t][:], biassb[:, ocs])
                                nc.gpsimd.dma_start(
                                    out_ap[t * 128 : (t + 1) * 128, ocs], osb[:]
                                )

            for oc in range(1, NOC):
                classic_sweep(oc)

    nc.compile()
    _CACHE["nc"] = nc
    return nc


def _prep_in_maps(x, weight, bias, router_V, router_U, router_W, experts_A, experts_B):
    FP8 = ml_dtypes.float8_e4m3
    xT_full = np.ascontiguousarray(
        x.reshape(TOKENS, D_IN).T.astype(np.float32)
    )  # [D_IN, TOKENS] fp32
    KB = KTB * 128
    xT_all = np.ascontiguousarray(xT_full[:KB].astype(BF16))        # bf16 part
    x8bT_all = np.ascontiguousarray((xT_full[KB:] / X8S).astype(FP8))  # fp8 part
    wT_full = weight.T.astype(np.float32)  # [D_IN, D_OUT]
    wT = np.ascontiguousarray(wT_full[:KB].astype(BF16))
    w8T = np.ascontiguousarray((wT_full[KB:] * X8S).astype(FP8))
    # projections pre-scaled x64 into fp8's normal range; the x64 is undone
    # by the activation scale (v, u) and by bcat's /64 (lora path)
    projT = np.ascontiguousarray(
        np.concatenate(
            [
                router_V.T,  # [D_IN, 128]
                router_U.T,  # [D_IN, 128]
                experts_A.transpose(1, 0, 2).reshape(D_IN, E * R),  # [D_IN, 128]
            ],
            axis=1,
        )
        * 64.0
    ).astype(FP8)
    xR8_all = xT_full.astype(FP8)
    rwrep = np.ascontiguousarray(np.repeat(router_W, R, axis=0).T.astype(BF16))
    bcat = np.ascontiguousarray((experts_B.reshape(E * R, D_OUT) / 64.0).astype(BF16))
    biasr = np.ascontiguousarray(
        np.broadcast_to(bias.astype(BF16), (128, D_OUT))
    )

    in_maps = []
    for c in range(N_CORES):
        ts = slice(c * T, (c + 1) * T)
        in_maps.append(
            {
                "xT": np.ascontiguousarray(xT_all[:, ts]),
                "x8bT": np.ascontiguousarray(x8bT_all[:, ts]),
                "xR8": np.ascontiguousarray(xR8_all[:, ts]),
                "wT": wT,
                "w8T": w8T,
                "projT": projT,
                "rwrep": rwrep,
                "bcat": bcat,
                "biasr": biasr,
            }
        )
    return in_maps


def _gather(results):
    out = np.concatenate(
        [np.asarray(results[c]["out"], dtype=np.float32) for c in range(N_CORES)],
        axis=0,
    )
    return out.reshape(B, N, D_OUT)


def kernel(x, weight, bias, router_V, router_U, router_W, experts_A, experts_B):
    import time
    from concourse.bass_utils import run_bass_kernel_spmd

    nc = _get_nc()
    in_maps = _prep_in_maps(
        x, weight, bias, router_V, router_U, router_W, experts_A, experts_B
    )
    last_err = None
    for attempt in range(3):
        try:
            res = run_bass_kernel_spmd(nc, in_maps, list(range(N_CORES)))
            return _gather(res.results)
        except Exception as e:  # transient NRT device errors — retry
            last_err = e
            try:  # drop the (possibly wedged) PJRT device context
                import jax

                jax.clear_caches()
                clear = getattr(
                    getattr(getattr(jax, "extend", None), "backend", None),
                    "clear_backends",
                    None,
                ) or getattr(jax, "clear_backends", None)
                if clear is not None:
                    clear()
            except Exception:
                pass
            time.sleep(5 * (attempt + 1))
    raise last_err


def run_traced(x, weight, bias, router_V, router_U, router_W, experts_A, experts_B):
    """Correctness + HW timing run (profiled). Returns (out, exec_time_ns, trace)."""
    import concourse.bass_utils as bass_utils

    bass_utils.upload_artifacts = lambda tmpdir: tmpdir  # no fileshare here
    nc = _get_nc()
    in_maps = _prep_in_maps(
        x, weight, bias, router_V, router_U, router_W, experts_A, experts_B
    )
    res = bass_utils.run_bass_kernel_spmd(
        nc, in_maps, list(range(N_CORES)), trace=True
    )
    trace_path = None
    if res.instructions_and_trace is not None:
        trace_path = res.instructions_and_trace[1]
    return _gather(res.results), res.exec_time_ns, trace_path


# revision 16
# speedup vs baseline: 1.0605x; 1.0270x over previous
"""Trainium2 Bass kernel for ABMIL-MoE-LoRA linear layer.

Reference computation (B=4, N=2048, D_IN=D_OUT=4096, E=8, R=16, D_ATT=128):
    base = x @ W.T + bias
    v = tanh(x @ V.T); u = sigmoid(x @ U.T)
    rw = sigmoid((v*u) @ router_W.T)                    # [B,N,E]
    lora = x @ A_e  (per expert)                        # [B,N,E,R]
    out = base + sum_e rw[...,e] * (lora_e @ B_e)

Strategy: data-parallel over the B*N = 8192 tokens across 8 NeuronCores
(1024 tokens/core, weights replicated). Matmuls run on the TensorEngine
with fp32 PSUM accumulation. Host-side prep pre-transposes every operand
so the contraction dim lands on SBUF partitions.

Precision split: 22 of the 32 contraction k-tiles of the base matmul run
in bf16; the last 10 run as 5 fp8-e4m3 DoubleRow matmuls (2 k-tiles per
pass, half the PE time). The fp8 operands carry cancelling power-of-2
scales (x/8, W*8) so their partial products accumulate into the SAME
PSUM bank as the bf16 partials with no epilogue fixup. The router path
shares the same x/8 fp8 data (projections pre-scaled x512 compensate).
Measured rel err of the hybrid ~1.78e-2 vs the 2e-2 gate.

Schedule: the router/LoRA-down projections are interleaved into the first
two output-column sweeps (k-tile by k-tile, matching DMA arrival order) so
the TensorEngine never starves while x / weights stream in. Those two
sweeps accumulate base-matmul partials into SBUF (PSUM banks are the
scarce resource); later sweeps use the classic 8-bank PSUM accumulation
with the MoE up-projection matmul fused into the same accumulation group.
A burst of dummy matmuls on a memset tile right after the NEFF preamble
warms the PE HAM clock gate so real matmuls never run at K=4/8.

Self-contained: hardcodes all shapes; only imports installed packages.
"""

import numpy as np
import ml_dtypes

BF16 = ml_dtypes.bfloat16

# Problem shapes (hardcoded per spec)
B, N, D_IN, D_OUT = 4, 2048, 4096, 4096
E, R, D_ATT = 8, 16, 128
TOKENS = B * N            # 8192
N_CORES = 8
T = TOKENS // N_CORES     # 1024 tokens per core
KT = D_IN // 128          # 32 contraction k-tiles
KTB = 22                  # k-tiles 0..21 in bf16
KT8 = KT - KTB            # k-tiles 22..31 in fp8 DoubleRow
KP8 = KT8 // 2            # 5 DoubleRow passes
OC = 512                  # output-column chunk per PSUM bank
NOC = D_OUT // OC         # 8 o-chunks
TT = T // 128             # 8 token tiles per core
KH = 2                    # bf16 weight streamed in 2 k-halves
KHT = KTB // KH           # 11 bf16 k-tiles per half
X8S = 8.0                 # x scaled by 1/X8S, W by X8S for the fp8 split
N_WARMUP = 10             # dummy MMs to warm the PE HAM clock gate

_CACHE = {}


def _get_nc():
    if "nc" in _CACHE:
        return _CACHE["nc"]

    import concourse.tile as tile
    import concourse.mybir as mybir
    from concourse import bacc

    dt = mybir.dt
    AFT = mybir.ActivationFunctionType
    nc = bacc.Bacc("TRN2", target_bir_lowering=False, debug=False)

    xT = nc.declare_dram_parameter("xT", [KTB * 128, T], dt.bfloat16, isOutput=False)
    wT = nc.declare_dram_parameter("wT", [KTB * 128, D_OUT], dt.bfloat16, isOutput=False)
    w8T = nc.declare_dram_parameter("w8T", [KT8 * 128, D_OUT], dt.float8e4, isOutput=False)
    projT = nc.declare_dram_parameter("projT", [D_IN, 384], dt.float8e4, isOutput=False)
    xR8 = nc.declare_dram_parameter("xR8", [D_IN, T], dt.float8e4, isOutput=False)
    rwrep = nc.declare_dram_parameter("rwrep", [128, 128], dt.bfloat16, isOutput=False)
    bcat = nc.declare_dram_parameter("bcat", [E * R, D_OUT], dt.bfloat16, isOutput=False)
    biasr = nc.declare_dram_parameter("biasr", [128, D_OUT], dt.bfloat16, isOutput=False)
    out = nc.declare_dram_parameter("out", [T, D_OUT], dt.float32, isOutput=True)

    xT_ap, wT_ap, projT_ap, xR8_ap = xT.ap(), wT.ap(), projT.ap(), xR8.ap()
    w8T_ap = w8T.ap()
    rwrep_ap, bcat_ap, biasr_ap, out_ap = rwrep.ap(), bcat.ap(), biasr.ap(), out.ap()

    with tile.TileContext(nc) as tc:
        with (
            tc.tile_pool(name="xpool", bufs=1) as xpool,
            tc.tile_pool(name="wpool", bufs=3) as wpool,
            tc.tile_pool(name="w8pool", bufs=2) as w8pool,
            tc.tile_pool(name="w0pool", bufs=1) as w0pool,
            tc.tile_pool(name="const", bufs=1) as constp,
            tc.tile_pool(name="inter", bufs=1) as inter,
            tc.tile_pool(name="opool", bufs=3) as opool,
            tc.tile_pool(name="ps", bufs=8, space="PSUM") as psp,
        ):
            xsb = xpool.tile([128, KTB * T], dt.bfloat16, tag="xsb")
            vub = inter.tile([128, T], dt.bfloat16, tag="vub")
            rwb = inter.tile([128, T], dt.bfloat16, tag="rwb")
            wtb = inter.tile([128, T], dt.bfloat16, tag="wtb")

            def ps_tile(name):
                return psp.tile([128, 512], dt.float32, tag="ps", name=name)

            # ---- PE warmup: dummy matmuls on a memset tile so the HAM
            # clock gate reaches K=8/8 before the first data-dependent
            # matmul issues (~12us in, right when the first DMAs land).
            # WAW on the single psum tile keeps them serialized. ----
            wub = constp.tile([128, 512], dt.bfloat16, tag="wub")
            nc.vector.memset(wub[:], 1.0)
            wups = ps_tile("warmup")
            for _ in range(N_WARMUP):
                nc.tensor.matmul(wups[:], wub[:, 0:128], wub[:], start=True, stop=True)

            # ---- sweeps 0 and 1: router half-sweep h fused with the base
            # matmul for o-chunk 0, token-half h. The oc0 weight chunk stays
            # resident across both sweeps; each (t) runs one full PSUM
            # accumulation group (24 bf16 k-tiles + 4 fp8 DoubleRow passes
            # + the fused MoE finish matmul).
            # Pointwise DMA demand stays under the HBM limit so the
            # TensorEngine never starves while x streams in. ----
            w0sb = w0pool.tile([128, KTB * OC], dt.bfloat16, tag="w0sb")
            w8sb0 = w0pool.tile([128, KT8 * OC], dt.float8e4, tag="w8sb0")
            x8b = w0pool.tile([128, KT8 * T], dt.float8e4, tag="x8b")
            projsb = w0pool.tile([128, KT * 384], dt.float8e4, tag="projsb")
            xsb8 = w0pool.tile([128, KTB * 512], dt.float8e4, tag="xsb8")
            xT_r = xT_ap.rearrange("(a p) t -> p a t", p=128)
            wT_r = wT_ap.rearrange("(a p) o -> p a o", p=128)
            w8T_r = w8T_ap.rearrange("(a p) o -> p a o", p=128)
            projT_r = projT_ap.rearrange("(a p) c -> p a c", p=128)
            xsb_r = xsb.rearrange("p (a t) -> p a t", a=KTB)
            w0sb_r = w0sb.rearrange("p (a o) -> p a o", a=KTB)
            w8sb0_r = w8sb0.rearrange("p (a o) -> p a o", a=KT8)
            x8b_r = x8b.rearrange("p (a t) -> p a t", a=KT8)
            projsb_r = projsb.rearrange("p (a c) -> p a c", a=KT)
            xR8_r = xR8_ap.rearrange("(a p) t -> p a t", p=128)
            xsb8_r = xsb8.rearrange("p (a t) -> p a t", a=KTB)
            ocs0 = slice(0, OC)

            for h, trange in ((0, range(0, 4)), (1, range(4, 8))):
                # all DMAs for this sweep upfront, in consumption order and
                # batched 4 k-tiles per transfer (~0.6us engine issue cost
                # per DMA trigger). Aggregate HBM bandwidth is the binding
                # constraint early on, and each ring is FIFO — so keep ONE
                # serialized consumption-ordered stream on sync (scalar only
                # covers the first singles so sync's trigger-issue cost is
                # hidden); parallel bulk rings just steal bandwidth from the
                # urgent transfers.
                hs = slice(h * 512, (h + 1) * 512)
                if h == 0:
                    nc.sync.dma_start(xsb8_r[:, 0:2, :], xR8_r[:, 0:2, hs])
                    nc.scalar.dma_start(projsb_r[:, 0:2, :], projT_r[:, 0:2, :])
                    nc.gpsimd.dma_start(xsb8_r[:, 2:4, :], xR8_r[:, 2:4, hs])
                    rwrepsb = constp.tile([128, 128], dt.bfloat16, tag="rwrepsb")
                    nc.gpsimd.dma_start(rwrepsb[:], rwrep_ap[:])
                    for k in range(4, 8, 2):
                        ka = slice(k, k + 2)
                        nc.sync.dma_start(xsb8_r[:, ka, :], xR8_r[:, ka, hs])
                        nc.sync.dma_start(projsb_r[:, ka, :], projT_r[:, ka, :])
                    nc.scalar.dma_start(projsb_r[:, 2:4, :], projT_r[:, 2:4, :])
                    for k in range(0, 8, 2):
                        ka = slice(k, k + 2)
                        nc.scalar.dma_start(xsb_r[:, ka, hs], xT_r[:, ka, hs])
                        nc.scalar.dma_start(w0sb_r[:, ka, :], wT_r[:, ka, ocs0])
                    for k0 in range(8, KT, 4):
                        ka = slice(k0, k0 + 4)
                        kb = slice(k0, min(k0 + 4, KTB))
                        if k0 < KTB:
                            nc.sync.dma_start(xsb8_r[:, kb, :], xR8_r[:, kb, hs])
                        nc.sync.dma_start(projsb_r[:, ka, :], projT_r[:, ka, :])
                        if k0 < KTB:
                            nc.sync.dma_start(xsb_r[:, kb, hs], xT_r[:, kb, hs])
                            nc.sync.dma_start(w0sb_r[:, kb, :], wT_r[:, kb, ocs0])
                    # late, non-critical sync tail: fp8 base operands and
                    # constants consumed only in this sweep's ending phase,
                    # kept out of the bandwidth-saturated early stream.
                    nc.sync.dma_start(x8b_r[:, :, hs], xR8_r[:, KTB:KT, hs])
                    nc.sync.dma_start(w8sb0_r[:, :, :], w8T_r[:, :, ocs0])
                    bcatsb = constp.tile([128, D_OUT], dt.bfloat16, tag="bcatsb")
                    nc.sync.dma_start(bcatsb[:], bcat_ap[:])
                    biassb = constp.tile([128, D_OUT], dt.bfloat16, tag="biassb")
                    nc.sync.dma_start(biassb[:], biasr_ap[:])
                else:
                    for k0 in range(0, KTB, 4):
                        kb = slice(k0, min(k0 + 4, KTB))
                        nc.sync.dma_start(xsb8_r[:, kb, :], xR8_r[:, kb, hs])
                        nc.sync.dma_start(xsb_r[:, kb, hs], xT_r[:, kb, hs])
                    nc.sync.dma_start(x8b_r[:, :, hs], xR8_r[:, KTB:KT, hs])

                vps = ps_tile(f"vps{h}")
                ups = ps_tile(f"ups{h}")
                lps = ps_tile(f"lps{h}")
                pst = {t: ps_tile(f"pst0_{t}") for t in trange}
                DELAY = 6

                def base_mms(k, trange=trange, pst=pst):
                    for t in trange:
                        nc.tensor.matmul(
                            pst[t][:],
                            xsb[:, k * T + t * 128 : k * T + (t + 1) * 128],
                            w0sb[:, k * OC : (k + 1) * OC],
                            start=(k == 0),
                            stop=False,
                        )

                DR = mybir.MatmulPerfMode.DoubleRow
                for k in range(KT):
                    if k % 2 == 0:
                        kp = k // 2
                        st, sp = kp == 0, kp == KT // 2 - 1
                        kpair = slice(k, k + 2)
                        if k < KTB:
                            rx8 = xsb8_r[:, kpair, :]
                        else:
                            lp = k - KTB
                            rx8 = x8b_r[:, lp : lp + 2, hs]
                        pj = projsb_r[:, kpair, :]
                        nc.tensor.matmul(
                            vps[:], pj[:, :, 0:128], rx8,
                            start=st, stop=sp, perf_mode=DR,
                        )
                        nc.tensor.matmul(
                            ups[:], pj[:, :, 128:256], rx8,
                            start=st, stop=sp, perf_mode=DR,
                        )
                        nc.tensor.matmul(
                            lps[:], pj[:, :, 256:384], rx8,
                            start=st, stop=sp, perf_mode=DR,
                        )
                    if k >= DELAY and k - DELAY < KTB:
                        base_mms(k - DELAY)
                for k in range(KT - DELAY, KTB):
                    base_mms(k)

                # router epilogue for half h (the ABMIL gate is per-token
                # separable, so scores/gates/weighted-lora complete within
                # the half). The fp8 DoubleRow tails interleave around the
                # scores matmul to cover the scalar/vector latency chain.
                vtmp = inter.tile([128, 512], dt.float32, tag="vtmp", name=f"vtmp{h}")
                utmp = inter.tile([128, 512], dt.float32, tag="utmp", name=f"utmp{h}")
                nc.scalar.activation(vtmp[:], vps[:], AFT.Tanh, scale=1.0 / 64)
                nc.scalar.activation(utmp[:], ups[:], AFT.Sigmoid, scale=1.0 / 64)
                nc.vector.tensor_mul(vub[:, hs], vtmp[:], utmp[:])

                def dr_tail(t):
                    for p in range(KP8):
                        pr = slice(2 * p, 2 * p + 2)
                        nc.tensor.matmul(
                            pst[t][:],
                            x8b_r[:, pr, t * 128 : (t + 1) * 128],
                            w8sb0_r[:, pr, :],
                            start=False,
                            stop=False,
                            perf_mode=DR,
                        )

                ts_ = list(trange)
                dr_tail(ts_[0])
                dr_tail(ts_[1])
                sps = ps_tile(f"sps{h}")
                nc.tensor.matmul(sps[:], rwrepsb[:], vub[:, hs], start=True, stop=True)
                nc.scalar.activation(rwb[:, hs], sps[:], AFT.Sigmoid)
                nc.vector.tensor_mul(wtb[:, hs], lps[:], rwb[:, hs])
                dr_tail(ts_[2])
                dr_tail(ts_[3])
                for t in trange:
                    nc.tensor.matmul(
                        pst[t][:],
                        wtb[:, t * 128 : (t + 1) * 128],
                        bcatsb[:, ocs0],
                        start=False,
                        stop=True,
                    )
                    osb = opool.tile([128, 512], dt.float32, tag="osb")
                    nc.vector.tensor_add(osb[:], pst[t][:], biassb[:, ocs0])
                    nc.gpsimd.dma_start(
                        out_ap[t * 128 : (t + 1) * 128, ocs0], osb[:]
                    )

            # ---- sweeps 1..7: classic 8-bank PSUM accumulation with the
            # MoE up-projection fused into each group. sync carries only the
            # input weight stream (so prefetch never queues behind output
            # triggers); gpsimd carries the output stream. ----
            DRm = mybir.MatmulPerfMode.DoubleRow

            def classic_sweep(oc):
                ocs = slice(oc * OC, (oc + 1) * OC)
                pst = [None] * TT
                w8sb = w8pool.tile(
                    [128, KT8 * OC], dt.float8e4, tag="w8sb", name=f"w8sb{oc}"
                )
                w8sb_r = w8sb.rearrange("p (a o) -> p a o", a=KT8)
                nc.sync.dma_start(w8sb_r[:, :, :], w8T_r[:, :, ocs])
                for kh in range(KH):
                    wsb = wpool.tile(
                        [128, KHT * OC], dt.bfloat16, tag="wsb", name=f"wsb{oc}_{kh}"
                    )
                    wsb_r = wsb.rearrange("p (a o) -> p a o", a=KHT)
                    for kk0 in range(0, KHT, 4):
                        kk1 = min(kk0 + 4, KHT)
                        nc.sync.dma_start(
                            wsb_r[:, kk0:kk1, :],
                            wT_r[:, kh * KHT + kk0 : kh * KHT + kk1, ocs],
                        )
                    for t in range(TT):
                        if kh == 0:
                            pst[t] = ps_tile(f"pst{oc}_{t}")
                            # fp8 DoubleRow passes open the group (their
                            # small weight DMA lands before the bf16 bulk)
                            for p in range(KP8):
                                pr = slice(2 * p, 2 * p + 2)
                                nc.tensor.matmul(
                                    pst[t][:],
                                    x8b_r[:, pr, t * 128 : (t + 1) * 128],
                                    w8sb_r[:, pr, :],
                                    start=(p == 0),
                                    stop=False,
                                    perf_mode=DRm,
                                )
                        for kk in range(KHT):
                            k = kh * KHT + kk
                            nc.tensor.matmul(
                                pst[t][:],
                                xsb[:, k * T + t * 128 : k * T + (t + 1) * 128],
                                wsb[:, kk * OC : (kk + 1) * OC],
                                start=False,
                                stop=False,
                            )
                        if kh == KH - 1:
                            nc.tensor.matmul(
                                pst[t][:],
                                wtb[:, t * 128 : (t + 1) * 128],
                                bcatsb[:, ocs],
                                start=False,
                                stop=True,
                            )
                            osb = opool.tile([128, 512], dt.float32, tag="osb")
                            if oc == NOC - 1 and t == TT - 1:
                                # final tile: split add+store across two DMA
                                # queues to halve the tail latency
                                o0 = oc * OC
                                nc.vector.tensor_add(
                                    osb[:, 0:256], pst[t][:, 0:256],
                                    biassb[:, o0 : o0 + 256],
                                )
                                nc.sync.dma_start(
                                    out_ap[t * 128 :, o0 : o0 + 256],
                                    osb[:, 0:256],
                                )
                                nc.vector.tensor_add(
                                    osb[:, 256:512], pst[t][:, 256:512],
                                    biassb[:, o0 + 256 : o0 + 512],
                                )
                                nc.scalar.dma_start(
                                    out_ap[t * 128 :, o0 + 256 : o0 + 512],
                                    osb[:, 256:512],
                                )
                            else:
                                nc.vector.tensor_add(osb[:], pst[t][:], biassb[:, ocs])
                                nc.gpsimd.dma_start(
                                    out_ap[t * 128 : (t + 1) * 128, ocs], osb[:]
                                )

            for oc in range(1, NOC):
                classic_sweep(oc)

    nc.compile()
    _CACHE["nc"] = nc
    return nc


def _prep_in_maps(x, weight, bias, router_V, router_U, router_W, experts_A, experts_B):
    FP8 = ml_dtypes.float8_e4m3
    xT_full = np.ascontiguousarray(
        x.reshape(TOKENS, D_IN).T.astype(np.float32)
    )  # [D_IN, TOKENS] fp32
    KB = KTB * 128
    xT_all = np.ascontiguousarray(xT_full[:KB].astype(BF16))        # bf16 part
    wT_full = weight.T.astype(np.float32)  # [D_IN, D_OUT]
    wT = np.ascontiguousarray(wT_full[:KB].astype(BF16))
    w8T = np.ascontiguousarray((wT_full[KB:] * X8S).astype(FP8))
    # xR8 carries ALL of x at the fp8 split's 1/8 scale; the router path
    # compensates via projections pre-scaled x(64*X8S), undone by the
    # activation scale (v, u) and by bcat's /64 (lora path)
    projT = np.ascontiguousarray(
        np.concatenate(
            [
                router_V.T,  # [D_IN, 128]
                router_U.T,  # [D_IN, 128]
                experts_A.transpose(1, 0, 2).reshape(D_IN, E * R),  # [D_IN, 128]
            ],
            axis=1,
        )
        * (64.0 * X8S)
    ).astype(FP8)
    xR8_all = (xT_full / X8S).astype(FP8)
    rwrep = np.ascontiguousarray(np.repeat(router_W, R, axis=0).T.astype(BF16))
    bcat = np.ascontiguousarray((experts_B.reshape(E * R, D_OUT) / 64.0).astype(BF16))
    biasr = np.ascontiguousarray(
        np.broadcast_to(bias.astype(BF16), (128, D_OUT))
    )

    in_maps = []
    for c in range(N_CORES):
        ts = slice(c * T, (c + 1) * T)
        in_maps.append(
            {
                "xT": np.ascontiguousarray(xT_all[:, ts]),
                "xR8": np.ascontiguousarray(xR8_all[:, ts]),
                "wT": wT,
                "w8T": w8T,
                "projT": projT,
                "rwrep": rwrep,
                "bcat": bcat,
                "biasr": biasr,
            }
        )
    return in_maps


def _gather(results):
    out = np.concatenate(
        [np.asarray(results[c]["out"], dtype=np.float32) for c in range(N_CORES)],
        axis=0,
    )
    return out.reshape(B, N, D_OUT)


def kernel(x, weight, bias, router_V, router_U, router_W, experts_A, experts_B):
    import time
    from concourse.bass_utils import run_bass_kernel_spmd

    nc = _get_nc()
    in_maps = _prep_in_maps(
        x, weight, bias, router_V, router_U, router_W, experts_A, experts_B
    )
    last_err = None
    for attempt in range(3):
        try:
            res = run_bass_kernel_spmd(nc, in_maps, list(range(N_CORES)))
            return _gather(res.results)
        except Exception as e:  # transient NRT device errors — retry
            last_err = e
            try:  # drop the (possibly wedged) PJRT device context
                import jax

                jax.clear_caches()
                clear = getattr(
                    getattr(getattr(jax, "extend", None), "backend", None),
                    "clear_backends",
                    None,
                ) or getattr(jax, "clear_backends", None)
                if clear is not None:
                    clear()
            except Exception:
                pass
            time.sleep(5 * (attempt + 1))
    raise last_err


def run_traced(x, weight, bias, router_V, router_U, router_W, experts_A, experts_B):
    """Correctness + HW timing run (profiled). Returns (out, exec_time_ns, trace)."""
    import concourse.bass_utils as bass_utils

    bass_utils.upload_artifacts = lambda tmpdir: tmpdir  # no fileshare here
    nc = _get_nc()
    in_maps = _prep_in_maps(
        x, weight, bias, router_V, router_U, router_W, experts_A, experts_B
    )
    res = bass_utils.run_bass_kernel_spmd(
        nc, in_maps, list(range(N_CORES)), trace=True
    )
    trace_path = None
    if res.instructions_and_trace is not None:
        trace_path = res.instructions_and_trace[1]
    return _gather(res.results), res.exec_time_ns, trace_path
